# revision 5
# baseline (speedup 1.0000x reference)
"""Causal self-attention (B=4, T=2048, C=1024, H=16) on 8 trn2 NeuronCores.

Sharding: data-parallel over batch (4) x tensor-parallel over heads (2 groups
of 8). Core c handles batch c//2, head-group c%2. Each core computes its
partial output projection (W_proj rows of its heads); the host sums the two
head-group partials per batch and adds b_proj.

Per-core kernel (all matmuls fp32r = full PE rate, ~1e-4 rel err):
  phase 1: QKV projection. Q^T,K^T stored [head-dim, T] (j on partitions),
           V stored [T, heads, 65] with a ones column (65th) so the attention
           AV matmul produces the softmax denominator for free.
  phase 2: per head, per key-block kb of 128: S^T[k,q] = K^T_blk.T @ Q^T
           (causal: only q >= kb*128), exp on ACT (scale=1/sqrt(64) folded
           in), diag-block mask on DVE, then y^T[65, q] += V1_kb.T @ P^T_kb
           accumulated in PSUM over kb. Softmax division done after the kb
           loop: reciprocal of the denominator row, broadcast to 128
           partitions via a K=1 matmul, multiply + evict on DVE.
  phase 3: partial out[t, e] = sum_j y^T[j, t] * W_proj[j, e].
"""

from contextlib import ExitStack

import numpy as np

import concourse.bass as bass
import concourse.mybir as mybir
import concourse.tile as tile
from concourse import bacc
from concourse.bass_utils import run_bass_kernel_spmd
from concourse.masks import make_upper_triangular

P = 128
T = 2048
C = 1024
HG = 8          # heads per core
D = 64
DG = HG * D     # 512
KT = C // P     # 8 contraction tiles for the qkv projection
JT = DG // P    # 4 row-tiles of Q^T/K^T (2 heads each)
TB = T // P     # 16 t/key blocks
QC = T // 512   # 4 512-wide column chunks
f32 = mybir.dt.float32
f32r = mybir.dt.float32r
EXP = mybir.ActivationFunctionType.Exp


def _emit(nc, tc, rep, xT, wq, wk, wv, wp, bq, bk, bv, out):
    with ExitStack() as es:
        pfx = f"r{rep}_"
        p_const = es.enter_context(tc.tile_pool(name=pfx + "const", bufs=1))
        mask32 = p_const.tile([P, P], f32)
        make_upper_triangular(nc, mask32[:], val=1.0, diag=True)
        mask_ut = p_const.tile([P, P], f32r)
        nc.vector.tensor_copy(mask_ut[:], mask32[:])
        ones32 = p_const.tile([P, P], f32)
        nc.gpsimd.memset(ones32[:], 1.0)
        ones65 = p_const.tile([65, P], f32r)
        nc.vector.tensor_copy(ones65[:], ones32[0:65, :])
        bq_t = p_const.tile([P, JT], f32)
        nc.sync.dma_start(out=bq_t[:], in_=bq.rearrange("(j p) -> p j", p=P))
        bk_t = p_const.tile([P, JT], f32)
        nc.sync.dma_start(out=bk_t[:], in_=bk.rearrange("(j p) -> p j", p=P))
        bv_row = p_const.tile([1, DG], f32r)
        nc.sync.dma_start(out=bv_row[:], in_=bv.rearrange("(o n) -> o n", o=1))

        # ---------------- persistent attention tensors -------------------
        p_qkv = es.enter_context(tc.tile_pool(name=pfx + "qkv", bufs=1))
        V1 = p_qkv.tile([P, TB, HG, 65], f32r)  # V with ones column, [t, h, d|1]
        Kt = p_qkv.tile([P, JT, T], f32r)       # K^T: [j-dim, T]
        Qt = p_qkv.tile([P, JT, T], f32r)       # Q^T

        # ---------------- phase 1: qkv projection ------------------------
        with ExitStack() as es1:
            p_w1 = es1.enter_context(tc.tile_pool(name=pfx + "w1", bufs=1))
            p_x = es1.enter_context(tc.tile_pool(name=pfx + "x", bufs=2))
            ps1 = es1.enter_context(
                tc.tile_pool(name=pfx + "ps1", bufs=4, space="PSUM"))

            wq_t = p_w1.tile([P, KT, DG], f32r)
            nc.sync.dma_start(out=wq_t[:], in_=wq.rearrange("(k p) n -> p k n", p=P))
            wk_t = p_w1.tile([P, KT, DG], f32r)
            nc.sync.dma_start(out=wk_t[:], in_=wk.rearrange("(k p) n -> p k n", p=P))
            wv_t = p_w1.tile([P, KT, DG], f32r)
            nc.sync.dma_start(out=wv_t[:], in_=wv.rearrange("(k p) n -> p k n", p=P))

            # broadcast b_v to 128 partitions (K=1 matmul with a ones row)
            bv_ps = ps1.tile([P, DG], f32, tag="ps")
            nc.tensor.matmul(bv_ps[:], ones65[0:1, :], bv_row[:],
                             start=True, stop=True)
            bias_v = p_w1.tile([P, DG], f32)
            nc.vector.tensor_copy(bias_v[:], bv_ps[:])

            # ones column of V1 (65th dim entry per head)
            nc.vector.tensor_copy(
                V1[:, :, :, 64],
                ones32[:, :].rearrange("p (a b) -> p a b", a=TB))

            for qc in range(QC):
                xc = p_x.tile([P, KT, 512], f32r, tag="xc")
                nc.sync.dma_start(
                    out=xc[:],
                    in_=xT[:, qc * 512:(qc + 1) * 512].rearrange(
                        "(k p) t -> p k t", p=P))
                # V rows for these 4 t-blocks: out[t, j] = x @ Wv
                for t4 in range(4):
                    tb = qc * 4 + t4
                    ps = ps1.tile([P, DG], f32, tag="ps")
                    for c in range(KT):
                        nc.tensor.matmul(
                            ps[:], xc[:, c, t4 * P:(t4 + 1) * P],
                            wv_t[:, c], start=(c == 0), stop=(c == KT - 1))
                    nc.vector.tensor_add(
                        V1[:, tb, :, 0:64],
                        ps[:].rearrange("p (h d) -> p h d", h=HG),
                        bias_v[:].rearrange("p (h d) -> p h d", h=HG))
                # K^T and Q^T columns for this 512-chunk of t
                for j in range(JT):
                    ps = ps1.tile([P, 512], f32, tag="ps")
                    for c in range(KT):
                        nc.tensor.matmul(
                            ps[:], wk_t[:, c, j * P:(j + 1) * P],
                            xc[:, c], start=(c == 0), stop=(c == KT - 1))
                    nc.vector.tensor_scalar_add(
                        Kt[:, j, qc * 512:(qc + 1) * 512], ps[:], bk_t[:, j:j + 1])
                for j in range(JT):
                    ps = ps1.tile([P, 512], f32, tag="ps")
                    for c in range(KT):
                        nc.tensor.matmul(
                            ps[:], wq_t[:, c, j * P:(j + 1) * P],
                            xc[:, c], start=(c == 0), stop=(c == KT - 1))
                    nc.vector.tensor_scalar_add(
                        Qt[:, j, qc * 512:(qc + 1) * 512], ps[:], bq_t[:, j:j + 1])

        # ---------------- phase 2: attention ------------------------------
        p_y = es.enter_context(tc.tile_pool(name=pfx + "y", bufs=1))
        Yt = p_y.tile([P, JT, T], f32r)         # y^T (normalized), [j-dim, T]
        p_wp = es.enter_context(tc.tile_pool(name=pfx + "wp", bufs=1))
        wp_t = p_wp.tile([P, JT, C], f32r)
        nc.sync.dma_start(out=wp_t[:], in_=wp.rearrange("(k p) n -> p k n", p=P))

        with ExitStack() as es2:
            p_pt = es2.enter_context(tc.tile_pool(name=pfx + "pt", bufs=3))
            p_tmp = es2.enter_context(tc.tile_pool(name=pfx + "ytmp", bufs=2))
            p_dn = es2.enter_context(tc.tile_pool(name=pfx + "dn", bufs=2))
            ps_s = es2.enter_context(
                tc.tile_pool(name=pfx + "ps_s", bufs=2, space="PSUM"))
            ps_y = es2.enter_context(
                tc.tile_pool(name=pfx + "ps_y", bufs=1, space="PSUM"))

            for h in range(HG):
                j, hp = divmod(h, 2)
                pb = hp * 64            # partition base of this head in Kt/Qt
                yt_ps = ps_y.tile([65, T], f32, tag="y")
                for kb in range(TB):
                    q0 = kb * P
                    # pieces of <=1024 score columns (2 psum banks each)
                    for p_off in range(q0, T, 1024):
                        plen = min(1024, T - p_off)
                        s_ps = ps_s.tile([P, 1024], f32, tag="s")
                        for c_off in range(p_off, p_off + plen, 512):
                            clen = min(512, p_off + plen - c_off)
                            nc.tensor.matmul(
                                s_ps[:, c_off - p_off:c_off - p_off + clen],
                                Kt[pb:pb + 64, j, q0:q0 + P],
                                Qt[pb:pb + 64, j, c_off:c_off + clen],
                                start=True, stop=True)
                        pt = p_pt.tile([P, 1024], f32r, tag="pt")
                        nc.scalar.activation(pt[:, :plen], s_ps[:, :plen],
                                             EXP, scale=0.125)
                        if p_off == q0:
                            # causal mask inside the diagonal 128x128 block
                            nc.vector.tensor_mul(pt[:, 0:P], pt[:, 0:P], mask_ut[:])
                        # AV accumulate, chunks aligned to the global 512 grid
                        c_off = p_off
                        while c_off < p_off + plen:
                            cell = c_off // 512
                            c_end = min((cell + 1) * 512, p_off + plen)
                            nc.tensor.matmul(
                                yt_ps[0:65, c_off:c_end],
                                V1[:, kb, h],
                                pt[:, c_off - p_off:c_end - p_off],
                                start=(kb == 0), stop=(kb == 4 * cell + 3))
                            c_off = c_end
                # softmax denominator -> reciprocal -> broadcast -> scale
                dn = p_dn.tile([65, T], f32r, tag="dn")
                with nc.allow_low_precision(reason="softmax 1/denom in fp32r"):
                    nc.vector.reciprocal(dn[64:65, :], yt_ps[64:65, :])
                for m2 in range(2):
                    bc_ps = ps_s.tile([P, 1024], f32, tag="s")
                    for m in range(2):
                        sl = slice(m * 512, (m + 1) * 512)
                        nc.tensor.matmul(
                            bc_ps[:, sl], ones65[64:65, :],
                            dn[64:65, m2 * 1024 + m * 512:
                               m2 * 1024 + (m + 1) * 512],
                            start=True, stop=True)
                    sl_t = slice(m2 * 1024, (m2 + 1) * 1024)
                    if hp == 0:
                        dst = Yt[0:64, j, sl_t]
                        nc.vector.tensor_copy(dst, yt_ps[0:64, sl_t])
                        nc.vector.tensor_mul(dst, dst, bc_ps[0:64, :])
                    else:
                        y_tmp = p_tmp.tile([64, 1024], f32r, tag="yt")
                        nc.vector.tensor_copy(y_tmp[:], yt_ps[0:64, sl_t])
                        nc.vector.tensor_mul(y_tmp[:], y_tmp[:], bc_ps[0:64, :])
                        nc.sync.dma_start(out=Yt[64:128, j, sl_t], in_=y_tmp[:])

        # ---------------- phase 3: output projection ----------------------
        with ExitStack() as es3:
            p_o = es3.enter_context(tc.tile_pool(name=pfx + "o", bufs=3))
            ps3 = es3.enter_context(
                tc.tile_pool(name=pfx + "ps3", bufs=3, space="PSUM"))
            for tb in range(TB):
                for ec in range(2):
                    ps = ps3.tile([P, 512], f32, tag="ps")
                    for kt in range(JT):
                        nc.tensor.matmul(
                            ps[:], Yt[:, kt, tb * P:(tb + 1) * P],
                            wp_t[:, kt, ec * 512:(ec + 1) * 512],
                            start=(kt == 0), stop=(kt == JT - 1))
                    o_sb = p_o.tile([P, 512], f32, tag="o")
                    nc.scalar.copy(o_sb[:], ps[:])
                    nc.sync.dma_start(
                        out=out[tb * P:(tb + 1) * P, ec * 512:(ec + 1) * 512],
                        in_=o_sb[:])


def build_program(reps=1):
    nc = bacc.Bacc(None, target_bir_lowering=False)
    xT = nc.declare_dram_parameter("xT", [C, T], f32r, isOutput=False)
    wq = nc.declare_dram_parameter("wq", [C, DG], f32r, isOutput=False)
    wk = nc.declare_dram_parameter("wk", [C, DG], f32r, isOutput=False)
    wv = nc.declare_dram_parameter("wv", [C, DG], f32r, isOutput=False)
    wp = nc.declare_dram_parameter("wp", [DG, C], f32r, isOutput=False)
    bq = nc.declare_dram_parameter("bq", [DG], f32, isOutput=False)
    bk = nc.declare_dram_parameter("bk", [DG], f32, isOutput=False)
    bv = nc.declare_dram_parameter("bv", [DG], f32r, isOutput=False)
    out = nc.declare_dram_parameter("out", [T, C], f32, isOutput=True)

    with tile.TileContext(nc) as tc:
        with nc.allow_low_precision(reason="fp32r attention kernel"):
            for rep in range(reps):
                _emit(nc, tc, rep, xT, wq, wk, wv, wp, bq, bk, bv, out)
    nc.compile()
    return nc


_PROGRAMS = {}


def _get_program(reps=1):
    if reps not in _PROGRAMS:
        _PROGRAMS[reps] = build_program(reps)
    return _PROGRAMS[reps]


def make_in_maps(x, W_attn, b_attn, W_proj):
    x = np.asarray(x, dtype=np.float32)
    W_attn = np.asarray(W_attn, dtype=np.float32)
    b_attn = np.asarray(b_attn, dtype=np.float32)
    W_proj = np.asarray(W_proj, dtype=np.float32)
    in_maps = []
    for c in range(8):
        b, g = divmod(c, 2)
        sl = slice(g * DG, (g + 1) * DG)
        in_maps.append({
            "xT": np.ascontiguousarray(x[b].T),
            "wq": np.ascontiguousarray(W_attn[:, 0 * C:1 * C][:, sl]),
            "wk": np.ascontiguousarray(W_attn[:, 1 * C:2 * C][:, sl]),
            "wv": np.ascontiguousarray(W_attn[:, 2 * C:3 * C][:, sl]),
            "wp": np.ascontiguousarray(W_proj[sl, :]),
            "bq": np.ascontiguousarray(b_attn[0 * C:1 * C][sl]),
            "bk": np.ascontiguousarray(b_attn[1 * C:2 * C][sl]),
            "bv": np.ascontiguousarray(b_attn[2 * C:3 * C][sl]),
        })
    return in_maps


def kernel(x, W_attn, b_attn, W_proj, b_proj, _reps=1):
    nc = _get_program(_reps)
    in_maps = make_in_maps(x, W_attn, b_attn, W_proj)
    res = run_bass_kernel_spmd(nc, in_maps, core_ids=list(range(8)))
    b_proj = np.asarray(b_proj, dtype=np.float32)
    out = np.empty((4, T, C), dtype=np.float32)
    for b in range(4):
        out[b] = res.results[2 * b]["out"] + res.results[2 * b + 1]["out"] + b_proj
    return out
